# revision 8
# baseline (speedup 1.0000x reference)
"""Trainium2 Bass kernel for nn_Conv_39333310497378 (nms_detection).

Reference computation:
  x [16384, 1, 41, 40] f32, W [9, 50, 1, 6, 40] f32
  36 sliding 6-row windows j (window j = rows j..j+5, section sec=j//4),
  out[b, j, o] = <x[b, rows j..j+5, :], W[sec, o]>  (240-elem dot)
  pots[b, sec, o] = max over h=j%4 of out[b, 4 sec+h, o]
  spks = (pots > 6.2) as 1.0/0.0.

Strategy (data parallel over batch, 8 cores x 2048 samples):
  Per batch tile of 128 samples the 36x50 output columns accumulate in
  PSUM (cols j*50+o, 1800 of 2048 across 4 banks).  x is chunked into 14
  three-row chunks of 120 elements (stride 120); each window is covered
  by 2-3 chunks (96 window-chunk pieces = 4800 moving columns per tile,
  vs 5850 for 128-aligned chunks).  Chunk c's x slice [120, 128] is the
  matmul stationary operand (fp8e3m4: 4x fast-weight-load), the banded
  per-chunk weights [120, <=400] (fp8e3m4) the moving operand.  One
  matmul per (chunk x psum bank); the first matmul touching a bank
  carries start=True, which clears the whole bank's has_written bits, so
  later matmuls overwrite-or-accumulate per element (no fresh/accumulate
  splitting).  VectorE max-pools h=4 via a two-level tensor_tensor max
  tree (PSUM pair-max -> bf16 SBUF pair-max), GpSimdE computes spks with
  a single is_gt, and outputs stream out per 4-tile group as contiguous
  [128, 4, 450] bf16 blocks on the scalar ring.
"""
import sys

import numpy as np

sys.path.insert(0, "/opt/trn_rl_repo")

import ml_dtypes  # noqa: E402

import concourse.bass as bass  # noqa: E402
import concourse.mybir as mybir  # noqa: E402
import concourse.tile as tile  # noqa: E402
from concourse import bacc  # noqa: E402
from concourse.bass_utils import run_bass_kernel_spmd  # noqa: E402

FP8 = mybir.dt.float8e3
BF16 = mybir.dt.bfloat16
F32 = mybir.dt.float32
NP_FP8 = ml_dtypes.float8_e3m4

B, ROWS, WIDTH = 16384, 41, 40
NSEC, OC, NJ = 9, 50, 36
THRESHOLD = 6.2
NCORES = 8
BC = B // NCORES            # 2048 samples per core
E = ROWS * WIDTH            # 1640 elements per sample
BT = 128                    # batch tile = psum partition dim
NT = BC // BT               # 16 batch tiles per core
GRP = 4                     # batch tiles per input DMA group
NG = NT // GRP              # 4 input groups
OG = 4                      # batch tiles per output DMA group
WLEN = 240                  # window length (6 rows x 40)

CLEN = 120                  # chunk length (3 rows)
CPAD = 128                  # chunk partition dim (8 zero rows: enables FWL)
CSTART = [120 * m for m in range(14)]
NCHUNK = len(CSTART)
EP = CSTART[-1] + CLEN      # 1680 padded elements per sample


def _plan():
    """Greedy min-cover of each window by chunks.

    Returns (cov, cwin, pieces):
      cov[c]   = list of (j, e0, e1) element ranges chunk c contributes
      cwin[c]  = (A, B) psum column window of chunk c
      pieces   = [(c, A, lo, hi, start, stop)] matmuls in emission order
    """
    cov = [[] for _ in range(NCHUNK)]
    for j in range(NJ):
        lo, hi = 40 * j, 40 * j + WLEN
        pos = lo
        while pos < hi:
            cands = [c for c, s in enumerate(CSTART) if s <= pos < s + CLEN]
            assert cands, f"window {j} uncovered at {pos}"
            best = max(cands, key=lambda c: CSTART[c] + CLEN)
            e1 = min(CSTART[best] + CLEN, hi)
            cov[best].append((j, pos, e1))
            pos = e1
    cwin = []
    for c in range(NCHUNK):
        js = [j for j, _, _ in cov[c]]
        assert js, f"chunk {c} unused"
        assert js == sorted(js) and js[-1] - js[0] == len(js) - 1, \
            f"chunk {c} windows not contiguous: {js}"
        cwin.append((OC * js[0], OC * (js[-1] + 1)))
    pieces = []
    seen = set()
    last = {}
    for c in range(NCHUNK):
        A, Bc = cwin[c]
        for k in range(A // 512, (Bc - 1) // 512 + 1):
            lo, hi = max(A, 512 * k), min(Bc, 512 * (k + 1))
            if lo >= hi:
                continue
            st = k not in seen
            if st:
                assert lo == 512 * k, f"bank {k} first piece lo={lo}"
                seen.add(k)
            pieces.append([c, A, lo, hi, st, False])
            last[k] = len(pieces) - 1
    for idx in last.values():
        pieces[idx][5] = True
    return cov, cwin, [tuple(p) for p in pieces]


def _build_wband(W):
    """Per-chunk banded weight tiles, concatenated -> ([120, TOTW], offsets)."""
    cov, cwin, _ = _plan()
    Wsq = np.asarray(W, np.float32)[:, :, 0]          # [9, 50, 6, 40]
    tiles, offs, off = [], [], 0
    for c in range(NCHUNK):
        A, Bc = cwin[c]
        wt = np.zeros((CPAD, Bc - A), np.float32)
        for (j, e0, e1) in cov[c]:
            es = np.arange(e0, e1)
            wt[es - CSTART[c], OC * j - A:OC * (j + 1) - A] = \
                Wsq[j // 4][:, es // 40 - j, es % 40].T
        tiles.append(wt)
        offs.append(off)
        off += Bc - A
    return np.concatenate(tiles, axis=1), offs


def _build_program(bc=BC):
    """One-core SPMD program for a [CLEN, NG, NCHUNK, GRP*BT] fp8 x shard."""
    _, cwin, pieces = _plan()
    totw = sum(b - a for a, b in cwin)
    woff = np.cumsum([0] + [b - a for a, b in cwin]).tolist()

    nc = bacc.Bacc(None)
    xT_d = nc.dram_tensor("xT", [CPAD, NG, NCHUNK, GRP * BT], FP8,
                          kind="ExternalInput")
    wb_d = nc.dram_tensor("Wb", [CPAD, totw], FP8, kind="ExternalInput")
    pots_d = nc.dram_tensor("pots", [NT // OG, BT, OG, OC * NSEC], BF16,
                            kind="ExternalOutput")
    spks_d = nc.dram_tensor("spks", [NT // OG, BT, OG, OC * NSEC], BF16,
                            kind="ExternalOutput")

    # group-0 x arrives in small sub-tiles so matmuls can start on the
    # first chunks while the rest is still in flight
    g0split = [(0, 2), (2, 3), (5, 3), (8, 3), (11, 3)]
    # weights arrive in two halves; the first matmuls only need half 0
    wsplit = woff[7]

    with tile.TileContext(nc) as tc:
        with (
            tc.tile_pool(name="w", bufs=1) as wpool,
            tc.tile_pool(name="x", bufs=2) as xpool,
            tc.tile_pool(name="t", bufs=2) as tpool,
            tc.tile_pool(name="out", bufs=2) as opool,
            tc.tile_pool(name="ps", bufs=2, space="PSUM") as pspool,
        ):
            wt0 = wpool.tile([CPAD, wsplit], FP8, tag="wt0", name="wt0")
            wt1 = wpool.tile([CPAD, totw - wsplit], FP8, tag="wt1",
                             name="wt1")
            nc.scalar.dma_start(wt0[:], wb_d[:, 0:wsplit])
            nc.scalar.dma_start(wt1[:], wb_d[:, wsplit:totw])
            x0 = []
            for i, (c0, nch) in enumerate(g0split):
                t = wpool.tile([CPAD, nch, GRP * BT], FP8, tag=f"x0_{i}",
                               name=f"x0_{i}")
                nc.sync.dma_start(t[:], xT_d[:, 0, c0:c0 + nch, :])
                x0.append(t)

            c2g = []
            for i, (c0, nch) in enumerate(g0split):
                c2g += [i] * nch
            po = sp = None
            xg = None
            for g in range(NG):
                if g > 0:
                    xg = xpool.tile([CPAD, NCHUNK, GRP * BT], FP8, tag="xg")
                    nc.sync.dma_start(xg[:], xT_d[:, g])
                for tl in range(GRP):
                    bt = g * GRP + tl
                    s = bt % OG
                    if s == 0:
                        po = opool.tile([BT, OG, OC * NSEC], BF16, tag="po")
                        sp = opool.tile([BT, OG, OC * NSEC], BF16, tag="sp")
                    ps = pspool.tile([BT, 2048], F32, tag="ps")
                    cp = tpool.tile([BT, 2 * OC * NSEC], BF16, tag="cp")
                    t2 = tpool.tile([BT, 2 * OC * NSEC], BF16, tag="t2")
                    for (c, A, lo, hi, st, stp) in pieces:
                        if g == 0:
                            gi = c2g[c]
                            lhsT = x0[gi][:, c - g0split[gi][0],
                                          tl * BT:(tl + 1) * BT]
                        else:
                            lhsT = xg[:, c, tl * BT:(tl + 1) * BT]
                        wt, wo = (wt0, woff[c]) if c < 7 else \
                            (wt1, woff[c] - wsplit)
                        nc.tensor.matmul(
                            ps[:, lo:hi], lhsT,
                            wt[:, wo + lo - A:wo + hi - A],
                            start=st, stop=stp, skip_group_check=True)
                    # pooling split across engines (DVE has one PSUM read
                    # port, so tensor_tensor straight from PSUM with two
                    # PSUM operands is illegal): ScalarE copies the h={2,3}
                    # half to SBUF bf16, VectorE maxes it against the
                    # h={0,1} half read from PSUM, reduces the pair, and
                    # thresholds for spks.
                    v = ps[:, :NJ * OC].rearrange("p (i h o) -> p i h o",
                                                  h=4, o=OC)
                    cpv = cp[:].rearrange("p (i h o) -> p i h o",
                                          h=2, o=OC)
                    nc.scalar.activation(cpv, v[:, :, 2:4, :],
                                         mybir.ActivationFunctionType.Copy)
                    t2v = t2[:].rearrange("p (i h o) -> p i h o",
                                          h=2, o=OC)
                    nc.vector.tensor_max(t2v, v[:, :, 0:2, :], cpv)
                    nc.vector.tensor_max(po[:, s, :], t2v[:, :, 0, :],
                                         t2v[:, :, 1, :])
                    nc.vector.tensor_scalar(
                        sp[:, s, :], po[:, s, :], float(THRESHOLD), None,
                        mybir.AluOpType.is_gt)
                    if s == OG - 1:
                        gi = bt // OG
                        nc.scalar.dma_start(pots_d[gi], po[:])
                        nc.gpsimd.dma_start(spks_d[gi], sp[:])
    nc.compile()
    return nc


_PROGRAM_CACHE = {}


def _get_program(bc=BC):
    if bc not in _PROGRAM_CACHE:
        _PROGRAM_CACHE[bc] = _build_program(bc)
    return _PROGRAM_CACHE[bc]


def _prep_inputs(x, W):
    wb, _ = _build_wband(W)
    wb8 = np.ascontiguousarray(wb).astype(NP_FP8)
    xf = np.asarray(x, np.float32).reshape(B, E)
    in_maps = []
    for ci in range(NCORES):
        xpad = np.zeros((BC, EP), np.float32)
        xpad[:, :E] = xf[ci * BC:(ci + 1) * BC]
        # [bc, EP] -> [NG, GRP*BT, NCHUNK, CLEN] -> [CLEN, NG, NCHUNK, GRP*BT]
        x4 = xpad.reshape(NG, GRP * BT, NCHUNK, CLEN).transpose(3, 0, 2, 1)
        x8 = np.zeros((CPAD, NG, NCHUNK, GRP * BT), NP_FP8)
        x8[:CLEN] = x4.astype(NP_FP8)
        in_maps.append({"xT": x8, "Wb": wb8})
    return in_maps


def kernel(x, W):
    nc = _get_program()
    in_maps = _prep_inputs(x, W)
    res = run_bass_kernel_spmd(nc, in_maps, list(range(NCORES)))
    pots_l, spks_l = [], []
    for r in res.results:
        # [NT//OG, BT, OG, 450] -> [NT//OG, OG, BT, 450] -> [BC, 9, 50]
        p4 = np.asarray(r["pots"]).astype(np.float32)
        s4 = np.asarray(r["spks"]).astype(np.float32)
        pots_l.append(p4.transpose(0, 2, 1, 3).reshape(BC, NSEC, OC))
        spks_l.append(s4.transpose(0, 2, 1, 3).reshape(BC, NSEC, OC))
    pots = np.concatenate(pots_l, axis=0).transpose(0, 2, 1).copy()
    spks = np.concatenate(spks_l, axis=0).transpose(0, 2, 1).copy()
    return pots.reshape(B, OC, NSEC, 1), spks.reshape(B, OC, NSEC, 1)
